# revision 1
# baseline (speedup 1.0000x reference)
"""Inverse STFT (nn_InverseSTFT) as a Bass/Tile kernel on 8 TRN2 NeuronCores.

Math
----
Reference computes, per batch b:
  full spectrum from one-sided stft via conjugate symmetry (F = 1024),
  ytmp[w, t] = sum_{f,c} full[f, t, c] * basis[f, w, c]          (IDFT)
  y = overlap_add(ytmp, hop=256), window-sum normalize, trim n_fft//2.

Folding the conjugate symmetry into the basis gives an exact K=1024 real
matmul (the imaginary basis rows for f=0 and f=512 are identically zero):
  rows 0..512   : A[f, w]  = cos-basis[f, w] + cos-basis[1024-f, w]   (f=1..511)
  rows 513..1023: Bm[f, w] = im-basis[f, w] - im-basis[1024-f, w]     (f=1..511)
computed with the reference's exact float32 angle arithmetic.

Since hop = 1024/4, write w = 256*j + r. Output sample n = 256*s + r:
  y[256 s + r] = sum_{j=0..3} sum_k basis[k, 256 j + r] * x[k, s - j]
The overlap-add is just PSUM accumulation over 4 frame-shifted matmuls.
Window-sum normalization = multiply by 1/(# valid j), which is 0.25 for
all output segments except s=2 (1/3), s=2000 (1/3), s=2001 (1/2), s=2002 (1).
Output keeps segments s = 2..2002 (trim = first 2 segments).

Sharding: pure data parallel, 2 batches per core.
"""

import numpy as np

import concourse.bass as bass
import concourse.mybir as mybir
from concourse.tile import TileContext
from concourse import bacc, bass_utils

N_FFT = 1024
HOP = 256
B = 16
NFREQ = 513
T = 2000
NCORES = 8
NB = B // NCORES          # batches per core
KC = 8                    # K chunks of 128 (K = 1024)
PAD_L = 3                 # left zero pad (j shifts up to 3)
TPAD = 2056               # 3 + 2000 + 53 (right pad covers last tile reads)
SEG = 2003                # total segments in un-trimmed output
OUT_SEGS = 2001           # segments s = 2..2002
NT = 16                   # s-tiles of 128 per batch (last has 81 valid rows)
OUT_LEN = OUT_SEGS * HOP  # 512256

F32 = mybir.dt.float32

# Matmul input dtype: bfloat16 halves stft/basis DMA traffic and enables
# fast weight load (FWL) on the PE; accumulation stays fp32 in PSUM.
# Validated rel-err vs reference: f32 1.6e-6, bf16 2.1e-3.
import os as _os

USE_BF16 = _os.environ.get("ISTFT_BF16", "1") == "1"
DT_IN = mybir.dt.bfloat16 if USE_BF16 else F32

import ml_dtypes

NP_IN = ml_dtypes.bfloat16 if USE_BF16 else np.float32


def _make_basis() -> np.ndarray:
    """(1024, 1024) folded basis, matching reference's float32 angle math."""
    f = np.arange(N_FFT, dtype=np.float32)
    w = np.arange(N_FFT, dtype=np.float32)
    a32 = np.float32(2.0 * np.pi / N_FFT)
    t1 = (a32 * f).astype(np.float32)
    ang = (t1[:, None] * w[None, :]).astype(np.float32)
    reb = (np.cos(ang).astype(np.float32) / np.float32(N_FFT)).astype(np.float32)
    imb = (-np.sin(ang).astype(np.float32) / np.float32(N_FFT)).astype(np.float32)
    A = np.empty((NFREQ, N_FFT), np.float32)
    A[0] = reb[0]
    A[512] = reb[512]
    A[1:512] = reb[1:512] + reb[1023:512:-1]
    Bm = (imb[1:512] - imb[1023:512:-1]).astype(np.float32)
    return np.concatenate([A, Bm], axis=0)


def _make_scales() -> np.ndarray:
    """(128, 2) per-partition wss fixup (on top of the 0.25 folded into basis).

    col 0 -> first s-tile (s = 2..129): s=2 has 3 frames -> 4/3.
    col 1 -> last s-tile (s = 1922..2002): s=2000 -> 4/3, 2001 -> 2, 2002 -> 4.
    """
    sc = np.ones((128, 2), np.float32)
    sc[0, 0] = np.float32(4.0) / np.float32(3.0)
    sc[78, 1] = np.float32(4.0) / np.float32(3.0)
    sc[79, 1] = 2.0
    sc[80, 1] = 4.0
    return sc


def _prep_x(stft: np.ndarray) -> np.ndarray:
    """(16,513,2000,2) f32 -> (16, KC, 128, TPAD) K-major, t zero-padded."""
    re = stft[:, :, :, 0]                  # (B, 513, T)
    im = stft[:, 1:512, :, 1]              # (B, 511, T)
    xk = np.concatenate([re, im], axis=1)  # (B, 1024, T)
    X = np.zeros((B, N_FFT, TPAD), np.float32)
    X[:, :, PAD_L : PAD_L + T] = xk
    return np.ascontiguousarray(X.reshape(B, KC, 128, TPAD))


def _build_nc() -> bass.Bass:
    nc = bacc.Bacc()
    x_in = nc.dram_tensor("x_in", [NB, KC, 128, TPAD], DT_IN, kind="ExternalInput")
    basis_in = nc.dram_tensor("basis_in", [KC, 128, N_FFT], DT_IN, kind="ExternalInput")
    scale_in = nc.dram_tensor("scale_in", [128, 2], F32, kind="ExternalInput")
    out = nc.dram_tensor("out", [NB, OUT_SEGS, HOP], F32, kind="ExternalOutput")

    with TileContext(nc) as tc:
        with (
            tc.tile_pool(name="xp", bufs=1) as x_pool,
            tc.tile_pool(name="bp", bufs=1) as b_pool,
            tc.tile_pool(name="sp", bufs=1) as s_pool,
            tc.tile_pool(name="ev", bufs=4) as ev_pool,
            tc.tile_pool(name="ps", bufs=4, space="PSUM") as psum_pool,
        ):
            # x chunks issue first on the Sync HWDGE queues (the first
            # matmul's critical path); basis + scale go via GpSimd so the
            # two DMA instruction streams issue in parallel.
            x_sb = [[None] * KC for _ in range(NB)]
            for b in range(NB):
                for kc in range(KC):
                    xt = x_pool.tile(
                        [128, TPAD], DT_IN, name=f"x{b}_{kc}", tag=f"x{b}_{kc}"
                    )
                    nc.sync.dma_start(xt[:, :], x_in[b, kc])
                    x_sb[b][kc] = xt

            basis_sb = []
            for kc in range(KC):
                bt = b_pool.tile([128, N_FFT], DT_IN, name=f"bas{kc}", tag=f"bas{kc}")
                nc.gpsimd.dma_start(bt[:, :], basis_in[kc])
                basis_sb.append(bt)

            scale_sb = s_pool.tile([128, 2], F32, name="scale_sb", tag="scale_sb")
            scale_wu = s_pool.tile([128, 2], F32, name="scale_wu", tag="scale_wu")
            nc.gpsimd.dma_start(scale_sb[:, :], scale_in[:, :])
            # ACT warm-up read of the scale table so later edge-tile
            # activations on ScalarE don't each need the DMA-sem wait.
            nc.scalar.copy(scale_wu[:, :], scale_sb[:, :])

            for b in range(NB):
                for st in range(NT):
                    s0 = 2 + 128 * st
                    psum = psum_pool.tile([128, HOP], F32, name="psum", tag="psum")
                    first = True
                    for kc in range(KC):
                        for j in range(4):
                            c0 = s0 - j + PAD_L
                            nc.tensor.matmul(
                                psum[:, :],
                                x_sb[b][kc][:, c0 : c0 + 128],
                                basis_sb[kc][:, HOP * j : HOP * (j + 1)],
                                start=first,
                                stop=(kc == KC - 1 and j == 3),
                            )
                            first = False
                    # basis is pre-scaled by 0.25 (the steady-state 1/wss);
                    # the two edge tiles apply a per-partition fixup scale
                    # via ScalarE's activation scale vector.
                    ev = ev_pool.tile([128, HOP], F32, name="ev", tag="ev")
                    if st == 0:
                        nc.scalar.mul(ev[:, :], psum[:, :], scale_sb[:, 0:1])
                    elif st == NT - 1:
                        nc.scalar.mul(ev[:, :], psum[:, :], scale_sb[:, 1:2])
                    else:
                        nc.vector.tensor_copy(ev[:, :], psum[:, :])
                    rows = min(128, SEG - s0)
                    nc.sync.dma_start(
                        out[b, 128 * st : 128 * st + rows, :], ev[:rows, :]
                    )
    nc.finalize()
    return nc


def _run(inputs: dict, trace: bool = False):
    stft = np.asarray(inputs["stft_matrix"], dtype=np.float32)
    X = np.ascontiguousarray(_prep_x(stft).astype(NP_IN))
    basis = np.ascontiguousarray(
        (_make_basis() * np.float32(0.25)).reshape(KC, 128, N_FFT).astype(NP_IN)
    )

    scales = _make_scales()
    in_maps = [
        {"x_in": X[NB * c : NB * (c + 1)], "basis_in": basis, "scale_in": scales}
        for c in range(NCORES)
    ]
    nc = _build_nc()
    res = bass_utils.run_bass_kernel_spmd(
        nc, in_maps, core_ids=list(range(NCORES)), trace=trace
    )
    out = np.concatenate(
        [res.results[c]["out"].reshape(NB, OUT_LEN) for c in range(NCORES)], axis=0
    )
    return out, res


def kernel(**inputs) -> np.ndarray:
    out, _ = _run(inputs, trace=False)
    return out



# revision 3
# speedup vs baseline: 2.0588x; 2.0588x over previous
"""Inverse STFT (nn_InverseSTFT) as a Bass/Tile kernel on 8 TRN2 NeuronCores.

Math
----
Reference: full spectrum via conjugate symmetry (F = 1024), IDFT to ytmp,
overlap_add(hop=256), window-sum normalize, trim n_fft//2.

This kernel folds BOTH the conjugate symmetry AND the overlap-add into the
basis. With w = 256j + r the DFT phase satisfies
    e^{i 2pi f (256j+r)/1024} = i^{f j} * e^{i 2pi f r/1024},
so the 4-frame overlap-add sum becomes a per-frequency 4-tap filter along
frames with coefficients i^{f j} in {1, i, -1, -i} (fixed per class
c = f mod 4), followed by ONE K=1024 matmul per output segment:
    y[256 s + r] = sum_f cosb[f,r]*U[f,s] + sinb[f,r]*V[f,s]
where (U, V) are shift-add combinations of the one-sided (re, im) frames.
This is 1024 MACs per output sample instead of 4096 (4x less PE work);
the combine runs on the Vector engine (16 wide-tile adds per batch) and
overlaps with DMA. Frequencies are packed by class so every combine op has
uniform coefficients across its 128 partitions:
    tiles 0-3: re classes c0(f<512),c1,c2,c3 ; tile 4: [re f=512; im c0] ;
    tiles 5-7: im classes c1,c2,c3.
Combines per class (s = output segment, x[t] zero outside 0..T-1):
    c0: U = x[s]+x[s-1]+x[s-2]+x[s-3]          (re->U, im->V, same taps)
    c2: U = x[s]-x[s-1]+x[s-2]-x[s-3]
    c1: U = D2re[s] - D2im[s-1], V = D2im[s] + D2re[s-1],  D2[s]=x[s]-x[s-2]
    c3: U = D2re[s] + D2im[s-1], V = D2im[s] - D2re[s-1]
Window-sum normalization: 0.25 folded into the basis; per-partition fixup
scales on the two edge s-tiles (3/2/1 valid frames).

All data bf16 (PSUM accumulates fp32); validated rel-err vs reference 3.7e-3.
Sharding: pure data parallel, 2 batches per core.
"""

import numpy as np
import ml_dtypes

import concourse.bass as bass
import concourse.mybir as mybir
from concourse.tile import TileContext
from concourse import bacc, bass_utils

N_FFT = 1024
HOP = 256
B = 16
NFREQ = 513
T = 2000
NCORES = 8
NB = B // NCORES          # batches per core
T2 = 2052                 # X cols: t = -1..2050 at col t+1; valid t=0..1999
S = 2048                  # U cols: s = 2..2049 at col s-2; out keeps s=2..2002
OUT_SEGS = 2001
OUT_LEN = OUT_SEGS * HOP  # 512256
KORDER = [0, 2, 4, 6, 1, 5, 3, 7]   # DMA/combine/matmul K-chunk order

F32 = mybir.dt.float32
BF16 = mybir.dt.bfloat16
NP_BF16 = ml_dtypes.bfloat16

_f = np.arange(NFREQ)
_IDX = {c: _f[_f % 4 == c] for c in range(4)}
_C0_RE = _IDX[0][:128]     # f = 0,4,...,508
_C0_IM = _IDX[0][1:-1]     # f = 4,...,508


def _prep_x(stft: np.ndarray) -> np.ndarray:
    """(16,513,2000,2) f32 -> (16, 8, 128, T2) bf16 class-packed, t zero-padded."""
    re, im = stft[..., 0], stft[..., 1]
    X = np.zeros((B, 8, 128, T2), np.float32)
    X[:, 0, :, 1:T + 1] = re[:, _C0_RE]
    X[:, 1, :, 1:T + 1] = re[:, _IDX[1]]
    X[:, 2, :, 1:T + 1] = re[:, _IDX[2]]
    X[:, 3, :, 1:T + 1] = re[:, _IDX[3]]
    X[:, 4, 0, 1:T + 1] = re[:, 512]
    X[:, 4, 1:128, 1:T + 1] = im[:, _C0_IM]
    X[:, 5, :, 1:T + 1] = im[:, _IDX[1]]
    X[:, 6, :, 1:T + 1] = im[:, _IDX[2]]
    X[:, 7, :, 1:T + 1] = im[:, _IDX[3]]
    return np.ascontiguousarray(X.astype(NP_BF16))


def _make_basis2() -> np.ndarray:
    """(8, 128, 256) basis tiles matching the class packing; 0.25 wss folded."""
    a32 = np.float32(2.0 * np.pi / N_FFT)
    fv = np.arange(NFREQ, dtype=np.float32)
    rv = np.arange(HOP, dtype=np.float32)
    t1 = (a32 * fv).astype(np.float32)
    ang = (t1[:, None] * rv[None, :]).astype(np.float32)
    w = np.full(NFREQ, 2.0, np.float32)
    w[0] = 1.0
    w[512] = 1.0
    Cb = (np.cos(ang) / np.float32(N_FFT)) * w[:, None] * np.float32(0.25)
    Sb = (-np.sin(ang) / np.float32(N_FFT)) * w[:, None] * np.float32(0.25)
    Bt = np.zeros((8, 128, HOP), np.float32)
    Bt[0] = Cb[_C0_RE]
    Bt[1] = Cb[_IDX[1]]
    Bt[2] = Cb[_IDX[2]]
    Bt[3] = Cb[_IDX[3]]
    Bt[4, 0] = Cb[512]
    Bt[4, 1:128] = Sb[_C0_IM]
    Bt[5] = Sb[_IDX[1]]
    Bt[6] = Sb[_IDX[2]]
    Bt[7] = Sb[_IDX[3]]
    return np.ascontiguousarray(Bt.astype(NP_BF16))


def _make_scales() -> np.ndarray:
    """(128, 2) per-partition wss fixup (on top of the 0.25 folded into basis).

    col 0 -> first s-tile (s = 2..129): s=2 has 3 frames -> 4/3.
    col 1 -> last s-tile (s = 1922..2049): s=2000 -> 4/3, 2001 -> 2, 2002 -> 4.
    """
    sc = np.ones((128, 2), np.float32)
    sc[0, 0] = np.float32(4.0) / np.float32(3.0)
    sc[78, 1] = np.float32(4.0) / np.float32(3.0)
    sc[79, 1] = 2.0
    sc[80, 1] = 4.0
    return sc


def _build_nc() -> bass.Bass:
    nc = bacc.Bacc()
    x_in = nc.dram_tensor("x_in", [NB, 8, 128, T2], BF16, kind="ExternalInput")
    basis_in = nc.dram_tensor("basis_in", [8, 128, HOP], BF16, kind="ExternalInput")
    scale_in = nc.dram_tensor("scale_in", [128, 2], F32, kind="ExternalInput")
    # out[b, p, h, j, r]: segment (h*1024 + j*128 + p) sample r; p on SBUF
    # partitions so each half's store is one DMA with 4KB contiguous rows.
    out = nc.dram_tensor("out", [NB, 128, 2, 8, HOP], BF16, kind="ExternalOutput")

    with TileContext(nc) as tc:
        with (
            tc.tile_pool(name="xp", bufs=1) as x_pool,
            tc.tile_pool(name="up", bufs=1) as u_pool,
            tc.tile_pool(name="scr", bufs=1) as scr_pool,
            tc.tile_pool(name="bp", bufs=1) as b_pool,
            tc.tile_pool(name="sp", bufs=1) as s_pool,
            tc.tile_pool(name="ev", bufs=1) as ev_pool,
            tc.tile_pool(name="ps", bufs=1, space="PSUM") as psum_pool,
        ):
            # X chunks on the Sync HWDGE queue (feeds the pipeline);
            # basis + scale via GpSimd so the streams issue in parallel.
            x_sb = [[None] * 8 for _ in range(NB)]
            for b in range(NB):
                for k in KORDER:
                    xt = x_pool.tile([128, T2], BF16, name=f"x{b}_{k}", tag=f"x{b}_{k}")
                    nc.sync.dma_start(xt[:, :], x_in[b, k])
                    x_sb[b][k] = xt

            basis_sb = []
            for k in range(8):
                bt = b_pool.tile([128, HOP], BF16, name=f"bas{k}", tag=f"bas{k}")
                nc.gpsimd.dma_start(bt[:, :], basis_in[k])
                basis_sb.append(bt)

            scale_sb = s_pool.tile([128, 2], F32, name="scale_sb", tag="scale_sb")
            scale_wu = s_pool.tile([128, 2], F32, name="scale_wu", tag="scale_wu")
            nc.gpsimd.dma_start(scale_sb[:, :], scale_in[:, :])
            # ACT warm-up read so edge-tile activations skip the table load.
            nc.scalar.copy(scale_wu[:, :], scale_sb[:, :])

            # 4-tap frame combine on DVE. U[k][:, cs] is the K-chunk k of the
            # combined spectrum for segment s = cs + 2.
            u_sb = [[None] * 8 for _ in range(NB)]
            for b in range(NB):
                X = x_sb[b]
                U = [
                    u_pool.tile([128, S], BF16, name=f"u{b}_{k}", tag=f"u{b}_{k}")
                    for k in range(8)
                ]
                u_sb[b] = U

                def winsum(uk, xk, sname, sign):
                    p = scr_pool.tile([128, T2], BF16, name=sname, tag=sname)
                    # P[s] = X[s] +/- X[s-1] at col s (s = 0..2049)
                    if sign > 0:
                        nc.vector.tensor_add(p[:, 0:2050], xk[:, 1:2051], xk[:, 0:2050])
                    else:
                        nc.vector.tensor_sub(p[:, 0:2050], xk[:, 1:2051], xk[:, 0:2050])
                    # U[cs] = P[cs+2] + P[cs]  (= P[s] + P[s-2])
                    nc.vector.tensor_add(uk[:, 0:S], p[:, 2:2050], p[:, 0:S])

                def d2(xk, sname):
                    d = scr_pool.tile([128, T2], BF16, name=sname, tag=sname)
                    # D[cd] = X[t=cd+1] - X[t=cd-1] at col cd (cd = s-1, s=1..2049)
                    nc.vector.tensor_sub(d[:, 0:2049], xk[:, 2:2051], xk[:, 0:2049])
                    return d

                winsum(U[0], X[0], "sP0", +1)
                winsum(U[2], X[2], "sM2", -1)
                winsum(U[4], X[4], "sP4", +1)
                winsum(U[6], X[6], "sM6", -1)
                d1 = d2(X[1], "sD1")
                d5 = d2(X[5], "sD5")
                # U1[cs] = D1[s] - D5[s-1] ; U5[cs] = D5[s] + D1[s-1]
                nc.vector.tensor_sub(U[1][:, 0:S], d1[:, 1:2049], d5[:, 0:S])
                nc.vector.tensor_add(U[5][:, 0:S], d5[:, 1:2049], d1[:, 0:S])
                d3 = d2(X[3], "sD3")
                d7 = d2(X[7], "sD7")
                nc.vector.tensor_add(U[3][:, 0:S], d3[:, 1:2049], d7[:, 0:S])
                nc.vector.tensor_sub(U[7][:, 0:S], d7[:, 1:2049], d3[:, 0:S])

            # Matmul: per half-batch, 8 live PSUM banks; K-sweeps in arrival
            # order so the PE starts as soon as the first U chunk is ready.
            for b in range(NB):
                for h in range(2):
                    pss = [
                        psum_pool.tile([128, HOP], F32, name=f"ps{i}", tag=f"ps{i}")
                        for i in range(8)
                    ]
                    for ki, k in enumerate(KORDER):
                        for sti in range(8):
                            st = 8 * h + sti
                            nc.tensor.matmul(
                                pss[sti][:, :],
                                u_sb[b][k][:, 128 * st : 128 * st + 128],
                                basis_sb[k][:, :],
                                start=(ki == 0),
                                stop=(ki == 7),
                            )
                    ev = ev_pool.tile([128, 8, HOP], BF16, name=f"ev{h}", tag=f"ev{h}")
                    for sti in range(8):
                        st = 8 * h + sti
                        if st == 0:
                            nc.scalar.mul(ev[:, sti, :], pss[sti][:, :], scale_sb[:, 0:1])
                        elif st == 15:
                            nc.scalar.mul(ev[:, sti, :], pss[sti][:, :], scale_sb[:, 1:2])
                        else:
                            nc.scalar.copy(ev[:, sti, :], pss[sti][:, :])
                    # one store per half on the ACT HWDGE queue (Sync is busy
                    # issuing X loads); 4KB contiguous per partition row.
                    nc.scalar.dma_start(out[b, :, h], ev[:, :, :])
    nc.finalize()
    return nc


def _run(inputs: dict, trace: bool = False):
    stft = np.asarray(inputs["stft_matrix"], dtype=np.float32)
    X = _prep_x(stft)
    basis = _make_basis2()
    scales = _make_scales()
    in_maps = [
        {"x_in": X[NB * c : NB * (c + 1)], "basis_in": basis, "scale_in": scales}
        for c in range(NCORES)
    ]
    nc = _build_nc()
    res = bass_utils.run_bass_kernel_spmd(
        nc, in_maps, core_ids=list(range(NCORES)), trace=trace
    )
    outs = []
    for c in range(NCORES):
        o = res.results[c]["out"]  # (NB, 128, 2, 8, HOP) bf16
        o = np.asarray(o, dtype=np.float32).transpose(0, 2, 3, 1, 4)  # b,h,j,p,r
        outs.append(o.reshape(NB, 2048 * HOP)[:, : OUT_LEN])
    return np.concatenate(outs, axis=0), res


def kernel(**inputs) -> np.ndarray:
    out, _ = _run(inputs, trace=False)
    return out


# revision 6
# speedup vs baseline: 2.1581x; 1.0482x over previous
"""Inverse STFT (nn_InverseSTFT) as a Bass/Tile kernel on 8 TRN2 NeuronCores.

Math
----
Reference: full spectrum via conjugate symmetry (F = 1024), IDFT to ytmp,
overlap_add(hop=256), window-sum normalize, trim n_fft//2.

This kernel folds BOTH the conjugate symmetry AND the overlap-add into the
basis. With w = 256j + r the DFT phase satisfies
    e^{i 2pi f (256j+r)/1024} = i^{f j} * e^{i 2pi f r/1024},
so the 4-frame overlap-add sum becomes a per-frequency 4-tap filter along
frames with coefficients i^{f j} in {1, i, -1, -i} (fixed per class
c = f mod 4), followed by ONE K=1024 matmul per output segment:
    y[256 s + r] = sum_f cosb[f,r]*U[f,s] + sinb[f,r]*V[f,s]
i.e. 1024 MACs per output sample instead of 4096 (4x less PE work).

Frequencies are packed by class c = f mod 4 so every combine has uniform
taps across its 128 partitions:
    tiles 0-3: re classes c0(f<512),c1,c2,c3 ; tile 4: [re f=512; im c0] ;
    tiles 5-7: im classes c1,c2,c3.
Every K-chunk k then combines as U_k[s] = G_k[s] +/- G_k[s-2] where G_k is a
single two-operand shift-add:
    k=0: G=X0[s]+X0[s-1] (+)   k=2: G=X2[s]-X2[s-1] (+)
    k=4: G=X4[s]+X4[s-1] (+)   k=6: G=X6[s]-X6[s-1] (+)
    k=1: G=X1[s]-X5[s-1] (-)   k=5: G=X5[s]+X1[s-1] (-)
    k=3: G=X3[s]+X7[s-1] (-)   k=7: G=X7[s]-X3[s-1] (-)
2 elementwise ops per chunk, 32 total, split DVE (22) / GpSimd (10) so the
combine hides under the input DMA; the PE consumes U chunks in arrival order
with full-batch K-sweeps over 8 PSUM banks of [128, 512] (2 s-tiles/bank).

Window-sum normalization: 0.25 folded into the basis; per-partition fixup
scales on the two edge s-tiles (3/2/1 valid frames).

All data bf16 (PSUM accumulates fp32); validated rel-err vs reference 3.7e-3.
Sharding: pure data parallel, 2 batches per core.
"""

import numpy as np
import ml_dtypes

import concourse.bass as bass
import concourse.mybir as mybir
from concourse.tile import TileContext
from concourse import bacc, bass_utils

N_FFT = 1024
HOP = 256
B = 16
NFREQ = 513
T = 2000
NCORES = 8
NB = B // NCORES          # batches per core
T2 = 2052                 # X cols: t = -1..2050 at col t+1; valid t=0..1999
S = 2048                  # U cols: s = 2..2049 at col s-2; out keeps s=2..2002
OUT_SEGS = 2001
OUT_LEN = OUT_SEGS * HOP  # 512256
DMAORDER = [0, 6, 4, 2, 1, 5, 3, 7]   # X-chunk DMA issue order
KORDER = [0, 6, 4, 2, 1, 5, 3, 7]     # matmul K-sweep order (= U arrival order)

F32 = mybir.dt.float32
BF16 = mybir.dt.bfloat16
NP_BF16 = ml_dtypes.bfloat16

_f = np.arange(NFREQ)
_IDX = {c: _f[_f % 4 == c] for c in range(4)}
_C0_RE = _IDX[0][:128]     # f = 0,4,...,508
_C0_IM = _IDX[0][1:-1]     # f = 4,...,508


def _prep_x(stft: np.ndarray) -> np.ndarray:
    """(16,513,2000,2) f32 -> (16, 8, 128, T2) bf16 class-packed, t zero-padded."""
    re, im = stft[..., 0], stft[..., 1]
    X = np.zeros((B, 8, 128, T2), np.float32)
    X[:, 0, :, 1:T + 1] = re[:, _C0_RE]
    X[:, 1, :, 1:T + 1] = re[:, _IDX[1]]
    X[:, 2, :, 1:T + 1] = re[:, _IDX[2]]
    X[:, 3, :, 1:T + 1] = re[:, _IDX[3]]
    X[:, 4, 0, 1:T + 1] = re[:, 512]
    X[:, 4, 1:128, 1:T + 1] = im[:, _C0_IM]
    X[:, 5, :, 1:T + 1] = im[:, _IDX[1]]
    X[:, 6, :, 1:T + 1] = im[:, _IDX[2]]
    X[:, 7, :, 1:T + 1] = im[:, _IDX[3]]
    return np.ascontiguousarray(X.astype(NP_BF16))


def _make_basis2() -> np.ndarray:
    """(8, 128, 256) basis tiles matching the class packing; 0.25 wss folded."""
    a32 = np.float32(2.0 * np.pi / N_FFT)
    fv = np.arange(NFREQ, dtype=np.float32)
    rv = np.arange(HOP, dtype=np.float32)
    t1 = (a32 * fv).astype(np.float32)
    ang = (t1[:, None] * rv[None, :]).astype(np.float32)
    w = np.full(NFREQ, 2.0, np.float32)
    w[0] = 1.0
    w[512] = 1.0
    Cb = (np.cos(ang) / np.float32(N_FFT)) * w[:, None] * np.float32(0.25)
    Sb = (-np.sin(ang) / np.float32(N_FFT)) * w[:, None] * np.float32(0.25)
    Bt = np.zeros((8, 128, HOP), np.float32)
    Bt[0] = Cb[_C0_RE]
    Bt[1] = Cb[_IDX[1]]
    Bt[2] = Cb[_IDX[2]]
    Bt[3] = Cb[_IDX[3]]
    Bt[4, 0] = Cb[512]
    Bt[4, 1:128] = Sb[_C0_IM]
    Bt[5] = Sb[_IDX[1]]
    Bt[6] = Sb[_IDX[2]]
    Bt[7] = Sb[_IDX[3]]
    return np.ascontiguousarray(Bt.astype(NP_BF16))


def _make_scales() -> np.ndarray:
    """(128, 2) per-partition wss fixup (on top of the 0.25 folded into basis).

    col 0 -> first s-tile (s = 2..129): s=2 has 3 frames -> 4/3.
    col 1 -> last s-tile (s = 1922..2049): s=2000 -> 4/3, 2001 -> 2, 2002 -> 4.
    """
    sc = np.ones((128, 2), np.float32)
    sc[0, 0] = np.float32(4.0) / np.float32(3.0)
    sc[78, 1] = np.float32(4.0) / np.float32(3.0)
    sc[79, 1] = 2.0
    sc[80, 1] = 4.0
    return sc


def _build_nc() -> bass.Bass:
    nc = bacc.Bacc()
    x_in = nc.dram_tensor("x_in", [NB, 8, 128, T2], BF16, kind="ExternalInput")
    basis_in = nc.dram_tensor("basis_in", [8, 128, HOP], BF16, kind="ExternalInput")
    scale_in = nc.dram_tensor("scale_in", [128, 2], F32, kind="ExternalInput")
    # out[b, p, st*256 + r]: segment (st*128 + p), sample r. One DMA per batch
    # with 8KB contiguous per partition row.
    out = nc.dram_tensor("out", [NB, 128, 16 * HOP], BF16, kind="ExternalOutput")

    with TileContext(nc) as tc:
        with (
            tc.tile_pool(name="xp", bufs=1) as x_pool,
            tc.tile_pool(name="up", bufs=1) as u_pool,
            tc.tile_pool(name="scr", bufs=1) as scr_pool,
            tc.tile_pool(name="bp", bufs=1) as b_pool,
            tc.tile_pool(name="sp", bufs=1) as s_pool,
            tc.tile_pool(name="ev", bufs=1) as ev_pool,
            tc.tile_pool(name="ps", bufs=1, space="PSUM") as psum_pool,
        ):
            # X chunks on the Sync HWDGE queue (feeds the pipeline); basis +
            # scale on the ACT HWDGE queue so both streams issue in parallel.
            x_sb = [[None] * 8 for _ in range(NB)]
            for b in range(NB):
                for k in DMAORDER:
                    xt = x_pool.tile([128, T2], BF16, name=f"x{b}_{k}", tag=f"x{b}_{k}")
                    nc.sync.dma_start(xt[:, :], x_in[b, k])
                    x_sb[b][k] = xt

            scale_sb = s_pool.tile([128, 2], F32, name="scale_sb", tag="scale_sb")
            scale_wu = s_pool.tile([128, 2], F32, name="scale_wu", tag="scale_wu")
            nc.scalar.dma_start(scale_sb[:, :], scale_in[:, :])
            basis_sb = [None] * 8
            for k in KORDER:
                bt = b_pool.tile([128, HOP], BF16, name=f"bas{k}", tag=f"bas{k}")
                nc.scalar.dma_start(bt[:, :], basis_in[k])
                basis_sb[k] = bt
            # ACT warm-up read so later activations skip the table load.
            nc.scalar.copy(scale_wu[:, :], scale_sb[:, :])

            # Frame combine, 2 ops per K-chunk: G then U = G[s] +/- G[s-2].
            # G[:, cq] pairs X[t=cq] with X[t=cq-1]; U[:, cs] is segment cs+2.
            # All on DVE: GpSimd 2-input elementwise contends with DVE for
            # SBUF ports (measured ~4x mutual slowdown), so it stays idle.
            u_sb = [[None] * 8 for _ in range(NB)]
            for b in range(NB):
                for k in range(8):
                    u_sb[b][k] = u_pool.tile(
                        [128, S], BF16, name=f"u{b}_{k}", tag=f"u{b}_{k}"
                    )

            def emit_g(b, k, ka, kb, g_add):
                g = scr_pool.tile([128, T2], BF16, name=f"g{k}", tag=f"g{k}")
                op = nc.vector.tensor_add if g_add else nc.vector.tensor_sub
                op(g[:, 0:2050], x_sb[b][ka][:, 1:2051], x_sb[b][kb][:, 0:2050])
                return g

            def emit_u(b, k, g, u_add):
                op = nc.vector.tensor_add if u_add else nc.vector.tensor_sub
                op(u_sb[b][k][:, 0:S], g[:, 2:2050], g[:, 0:S])

            # (src_a, src_b, g_is_add, u_is_add); c3 pair (k=3,7) needs X7 so
            # it runs last, matching the X DMA order.
            SPEC = {
                0: (0, 0, True, True),
                6: (6, 6, False, True),
                4: (4, 4, True, True),
                2: (2, 2, False, True),
                1: (1, 5, False, False),
                5: (5, 1, True, False),
                3: (3, 7, True, False),
                7: (7, 3, False, False),
            }
            for b in range(NB):
                for k in [0, 6, 4, 2, 1, 5, 3, 7]:
                    sa, sb_, ga, ua = SPEC[k]
                    g = emit_g(b, k, sa, sb_, ga)
                    emit_u(b, k, g, ua)

            # Matmul: full-batch K-sweeps; PSUM bank i holds s-tiles (2i, 2i+1).
            # A matmul 'start' zeroes the WHOLE bank, so only the very first
            # matmul touching a bank carries start=True; the upper half's first
            # accumulation lands on the bank-wide zero.
            for b in range(NB):
                pss = [
                    psum_pool.tile([128, 2 * HOP], F32, name=f"ps{i}", tag=f"ps{i}")
                    for i in range(8)
                ]
                for ki, k in enumerate(KORDER):
                    for i in range(8):
                        for hf in range(2):
                            st = 2 * i + hf
                            nc.tensor.matmul(
                                pss[i][:, HOP * hf : HOP * (hf + 1)],
                                u_sb[b][k][:, 128 * st : 128 * st + 128],
                                basis_sb[k][:, :],
                                start=(ki == 0 and hf == 0),
                                stop=(ki == 7),
                                skip_group_check=True,
                            )
                ev = ev_pool.tile([128, 16 * HOP], BF16, name=f"ev{b}", tag=f"ev{b}")
                # Evict: ACT for banks 0-3 (and all of batch 0 -- DVE is still
                # combining then); the last batch splits banks 4-7 onto DVE,
                # which is idle by that point, to shorten the tail.
                last = b == NB - 1
                for i in range(8):
                    lo, hi = 512 * i, 512 * i + 256
                    if i == 0:
                        nc.scalar.mul(ev[:, lo:hi], pss[i][:, 0:HOP], scale_sb[:, 0:1])
                        nc.scalar.copy(ev[:, hi : hi + 256], pss[i][:, HOP:])
                    elif i == 7:
                        eng = nc.vector if last else nc.scalar
                        if last:
                            eng.tensor_copy(ev[:, lo:hi], pss[i][:, 0:HOP])
                            eng.tensor_scalar_mul(
                                ev[:, hi : hi + 256], pss[i][:, HOP:], scale_sb[:, 1:2]
                            )
                        else:
                            eng.copy(ev[:, lo:hi], pss[i][:, 0:HOP])
                            eng.mul(
                                ev[:, hi : hi + 256], pss[i][:, HOP:], scale_sb[:, 1:2]
                            )
                    elif last and i >= 4:
                        nc.vector.tensor_copy(ev[:, lo : lo + 512], pss[i][:, :])
                    else:
                        nc.scalar.copy(ev[:, lo : lo + 512], pss[i][:, :])
                    if i == 3:
                        nc.scalar.dma_start(out[b][:, 0:2048], ev[:, 0:2048])
                nc.scalar.dma_start(out[b][:, 2048:4096], ev[:, 2048:4096])
    nc.finalize()
    return nc


def _run(inputs: dict, trace: bool = False):
    stft = np.asarray(inputs["stft_matrix"], dtype=np.float32)
    X = _prep_x(stft)
    basis = _make_basis2()
    scales = _make_scales()
    in_maps = [
        {"x_in": X[NB * c : NB * (c + 1)], "basis_in": basis, "scale_in": scales}
        for c in range(NCORES)
    ]
    nc = _build_nc()
    res = bass_utils.run_bass_kernel_spmd(
        nc, in_maps, core_ids=list(range(NCORES)), trace=trace
    )
    outs = []
    for c in range(NCORES):
        o = res.results[c]["out"]  # (NB, 128, 4096) bf16
        o = np.asarray(o, dtype=np.float32).reshape(NB, 128, 16, HOP)
        o = o.transpose(0, 2, 1, 3)  # (NB, st, p, r)
        outs.append(o.reshape(NB, 2048 * HOP)[:, :OUT_LEN])
    return np.concatenate(outs, axis=0), res


def kernel(**inputs) -> np.ndarray:
    out, _ = _run(inputs, trace=False)
    return out
